# revision 9
# baseline (speedup 1.0000x reference)
"""Trainium2 Bass kernel for nn_A2Attention (B=2, S=4096, H=1024, NH=16, hd=64).

Sharding: 8 cores = data-parallel over batch (2) x tensor-parallel over heads (4
groups of 4 heads). Each core computes QKV projection for its 4 heads, RMSNorm +
RoPE on Q/K, causal flash attention in transposed-score layout, and a partial
row-parallel o_proj output [4096, 1024]; the host sums the 4 partials per batch.

Self-contained: hardcodes shapes and builds/compiles the NEFF on first call.
"""

import os
import sys

for _p in ("/root/.axon_site", "/root/.axon_site/_ro/trn_rl_repo",
           "/root/.axon_site/_ro/pypackages"):
    if _p not in sys.path and os.path.isdir(_p):
        sys.path.insert(0, _p)

import numpy as np
import ml_dtypes

BF16 = ml_dtypes.bfloat16

H = 1024
NH = 16
HD = 64
NCORES = 8
HEADS_PER_CORE = 4
EPS = 1e-6


DEBUG_TAPS = False


def build(S=4096):
    """Build the per-core Bacc graph (SPMD: same graph on all 8 cores)."""
    import concourse.bass as bass
    import concourse.mybir as mybir
    from concourse import bacc, tile

    dt = mybir.dt
    AF = mybir.ActivationFunctionType
    NSC = S // 512          # s-chunks of 512
    NST = S // 128          # s-tiles of 128
    HT = H // 128           # h-tiles (contraction) = 8

    nc = bacc.Bacc("TRN2", target_bir_lowering=False)

    xt_d = nc.declare_dram_parameter("xt", [H, S], dt.bfloat16, isOutput=False)
    wq_d = nc.declare_dram_parameter("wq", [H, 256], dt.bfloat16, isOutput=False)
    wk_d = nc.declare_dram_parameter("wk", [H, 256], dt.bfloat16, isOutput=False)
    wv_d = nc.declare_dram_parameter("wv", [H, 256], dt.bfloat16, isOutput=False)
    wo_d = nc.declare_dram_parameter("wo", [256, H], dt.bfloat16, isOutput=False)
    cq_d = nc.declare_dram_parameter("cq", [128, S], dt.bfloat16, isOutput=False)
    sq_d = nc.declare_dram_parameter("sq", [128, S], dt.bfloat16, isOutput=False)
    ck_d = nc.declare_dram_parameter("ck", [128, S], dt.bfloat16, isOutput=False)
    sk_d = nc.declare_dram_parameter("sk", [128, S], dt.bfloat16, isOutput=False)
    out_d = nc.declare_dram_parameter("out", [S, H], dt.float32, isOutput=True)
    if DEBUG_TAPS:
        dbg = {nm: nc.declare_dram_parameter(nm, [128, S], dt.float32, isOutput=True)
               for nm in ("d_qhat", "d_khat", "d_ot")}
        dbg["d_v"] = nc.declare_dram_parameter("d_v", [128, 256], dt.float32, isOutput=True)

    with tile.TileContext(nc) as tc:
        with (
            tc.tile_pool(name="const", bufs=1) as cpool,
            tc.tile_pool(name="vpool", bufs=NST) as vpool,
            tc.tile_pool(name="xtp", bufs=16) as xtp,
            tc.tile_pool(name="qk", bufs=2) as qkpool,
            tc.tile_pool(name="otp", bufs=2) as otpool,
            tc.tile_pool(name="ptp", bufs=2) as ptpool,
            tc.tile_pool(name="tmp", bufs=2) as tmp,
            tc.tile_pool(name="ps", bufs=3, space="PSUM") as psp,
            tc.tile_pool(name="stp", bufs=2, space="PSUM") as stp,
            tc.tile_pool(name="zzp", bufs=1, space="PSUM") as zzp,
        ):
            # ---- constants -------------------------------------------------
            wq_sb = cpool.tile([128, HT * 256], dt.bfloat16)
            wk_sb = cpool.tile([128, HT * 256], dt.bfloat16)
            wv_sb = cpool.tile([128, HT * 256], dt.bfloat16)
            for ht in range(HT):
                nc.sync.dma_start(out=wq_sb[:, ht * 256:(ht + 1) * 256],
                                  in_=wq_d[ht * 128:(ht + 1) * 128, :])
                nc.sync.dma_start(out=wk_sb[:, ht * 256:(ht + 1) * 256],
                                  in_=wk_d[ht * 128:(ht + 1) * 128, :])
                nc.sync.dma_start(out=wv_sb[:, ht * 256:(ht + 1) * 256],
                                  in_=wv_d[ht * 128:(ht + 1) * 128, :])
            wo_sb = cpool.tile([128, 2 * H], dt.bfloat16)
            nc.sync.dma_start(out=wo_sb[:, 0:H], in_=wo_d[0:128, :])
            nc.sync.dma_start(out=wo_sb[:, H:2 * H], in_=wo_d[128:256, :])
            rope = {}
            for nm, d in (("cq", cq_d), ("sq", sq_d), ("ck", ck_d), ("sk", sk_d)):
                t = cpool.tile([128, S], dt.bfloat16, tag=f"rope_{nm}")
                nc.sync.dma_start(out=t[:], in_=d[:])
                rope[nm] = t
            # causal mask strip: strip[kk, x] = 1 if x >= kk + 384 else 0
            # slice at offset 384-128*t gives the [128,512] mask for diagonal
            # k-tile t (valid where q_local >= k_local + 128*t).
            strip = cpool.tile([128, 896], dt.bfloat16)
            nc.gpsimd.memset(strip[:], 1.0)
            nc.gpsimd.affine_select(
                out=strip[:], in_=strip[:],
                compare_op=mybir.AluOpType.is_ge, fill=0.0,
                base=-384, pattern=[[1, 896]], channel_multiplier=-1)
            ones1 = cpool.tile([128, 1], dt.bfloat16)
            nc.gpsimd.memset(ones1[:], 1.0)
            ones2 = cpool.tile([128, 33], dt.bfloat16)
            nc.gpsimd.memset(ones2[:], 0.0)
            nc.gpsimd.memset(ones2[0:64, 0:1], 1.0)
            nc.gpsimd.memset(ones2[64:128, 32:33], 1.0)
            epsb = cpool.tile([128, 1], dt.float32)
            nc.gpsimd.memset(epsb[:], EPS)

            # ---- V projection: v[s, c] for all 4 heads ---------------------
            v_tiles = []
            for st in range(NST):
                sc = st // 4
                if st % 4 == 0:
                    xts = []
                    for ht in range(HT):
                        xt_t = xtp.tile([128, 512], dt.bfloat16, tag="xt")
                        nc.sync.dma_start(
                            out=xt_t[:],
                            in_=xt_d[ht * 128:(ht + 1) * 128, sc * 512:(sc + 1) * 512])
                        xts.append(xt_t)
                v_ps = psp.tile([128, 256], dt.float32, tag="ps")
                for ht in range(HT):
                    nc.tensor.matmul(
                        v_ps[:],
                        xts[ht][:, (st % 4) * 128:(st % 4 + 1) * 128],
                        wv_sb[:, ht * 256:(ht + 1) * 256],
                        start=(ht == 0), stop=(ht == HT - 1))
                v_sb = vpool.tile([128, 256], dt.bfloat16, tag="v")
                nc.vector.tensor_copy(v_sb[:], v_ps[:])
                v_tiles.append(v_sb)
                if DEBUG_TAPS and st == 0:
                    vf = tmp.tile([128, 256], dt.float32, tag="dbgv")
                    nc.vector.tensor_copy(vf[:], v_sb[:])
                    nc.sync.dma_start(out=dbg["d_v"][:], in_=vf[:])

            ot_tiles = []
            for p in range(2):  # head pairs (2p, 2p+1)
                # ---- stage A: Q^T/K^T projection + rmsnorm + rope ----------
                qhat = qkpool.tile([128, S], dt.bfloat16, tag="qhat")
                khat = qkpool.tile([128, S], dt.bfloat16, tag="qhat")
                for sc in range(NSC):
                    xts = []
                    for ht in range(HT):
                        xt_t = xtp.tile([128, 512], dt.bfloat16, tag="xt")
                        nc.sync.dma_start(
                            out=xt_t[:],
                            in_=xt_d[ht * 128:(ht + 1) * 128, sc * 512:(sc + 1) * 512])
                        xts.append(xt_t)
                    for w_sb, hat, cn, sn in ((wq_sb, qhat, "cq", "sq"),
                                              (wk_sb, khat, "ck", "sk")):
                        qt_ps = psp.tile([128, 512], dt.float32, tag="ps")
                        for ht in range(HT):
                            nc.tensor.matmul(
                                qt_ps[:],
                                w_sb[:, ht * 256 + 128 * p: ht * 256 + 128 * (p + 1)],
                                xts[ht][:],
                                start=(ht == 0), stop=(ht == HT - 1))
                        # sum of squares per head -> rstd
                        qsq = tmp.tile([128, 512], dt.bfloat16, tag="qsq")
                        nc.scalar.activation(qsq[:], qt_ps[:], AF.Square)
                        ssq = zzp.tile([33, 512], dt.float32, tag="zz")
                        nc.tensor.matmul(ssq[:], ones2[:], qsq[:],
                                         start=True, stop=True)
                        rln_a = tmp.tile([1, 512], dt.float32, tag="rln_a")
                        rln_b = tmp.tile([1, 512], dt.float32, tag="rln_b")
                        nc.scalar.activation(rln_a[:], ssq[0:1, :], AF.Ln,
                                             bias=epsb[0:1, :], scale=1.0 / HD)
                        nc.scalar.activation(rln_b[:], ssq[32:33, :], AF.Ln,
                                             bias=epsb[0:1, :], scale=1.0 / HD)
                        rstd_a = tmp.tile([1, 512], dt.bfloat16, tag="rstd_a")
                        rstd_b = tmp.tile([1, 512], dt.bfloat16, tag="rstd_b")
                        nc.scalar.activation(rstd_a[:], rln_a[:], AF.Exp, scale=-0.5)
                        nc.scalar.activation(rstd_b[:], rln_b[:], AF.Exp, scale=-0.5)
                        rb = tmp.tile([128, 512], dt.bfloat16, tag="rb")
                        rbb = tmp.tile([64, 512], dt.bfloat16, tag="rbb")
                        nc.gpsimd.partition_broadcast(rb[0:64, :], rstd_a[:])
                        nc.gpsimd.partition_broadcast(rbb[:], rstd_b[:])
                        nc.sync.dma_start(out=rb[64:128, :], in_=rbb[:])
                        # rope: hat = (q*cos_g + qshift*sin_g) * rstd
                        csl = rope[cn][:, sc * 512:(sc + 1) * 512]
                        ssl = rope[sn][:, sc * 512:(sc + 1) * 512]
                        qrw = tmp.tile([128, 512], dt.bfloat16, tag="qrw")
                        nc.vector.tensor_copy(qrw[:], qt_ps[:])
                        t1 = tmp.tile([128, 512], dt.bfloat16, tag="t1")
                        nc.vector.tensor_mul(t1[:], qrw[:], csl)
                        qs = tmp.tile([128, 512], dt.bfloat16, tag="qs")
                        nc.sync.dma_start(out=qs[0:32, :], in_=qrw[32:64, :])
                        nc.sync.dma_start(out=qs[32:64, :], in_=qrw[0:32, :])
                        nc.sync.dma_start(out=qs[64:96, :], in_=qrw[96:128, :])
                        nc.sync.dma_start(out=qs[96:128, :], in_=qrw[64:96, :])
                        t2 = tmp.tile([128, 512], dt.bfloat16, tag="t2")
                        nc.vector.tensor_mul(t2[:], qs[:], ssl)
                        nc.vector.tensor_add(t1[:], t1[:], t2[:])
                        nc.vector.tensor_mul(hat[:, sc * 512:(sc + 1) * 512],
                                             t1[:], rb[:])

                if DEBUG_TAPS and p == 0:
                    for nm, t_ in (("d_qhat", qhat), ("d_khat", khat)):
                        for scd in range(NSC):
                            tf = tmp.tile([128, 512], dt.float32, tag="dbgf")
                            nc.vector.tensor_copy(tf[:], t_[:, scd*512:(scd+1)*512])
                            nc.sync.dma_start(out=dbg[nm][:, scd*512:(scd+1)*512], in_=tf[:])
                # ---- stage B: causal flash attention -----------------------
                ot = otpool.tile([128, S], dt.bfloat16, tag="ot")
                ot_tiles.append(ot)
                for qc in range(NSC):
                    nkt = 4 * (qc + 1)
                    av = psp.tile([128, 512], dt.float32, tag="ps")
                    zz = zzp.tile([64, 512], dt.float32, tag="zz")
                    for kt in range(nkt):
                        st2 = stp.tile([128, 1024], dt.float32, tag="st")
                        nc.tensor.matmul(
                            st2[:, 0:512],
                            khat[0:64, kt * 128:(kt + 1) * 128],
                            qhat[0:64, qc * 512:(qc + 1) * 512],
                            start=True, stop=True, tile_position=(0, 0))
                        nc.tensor.matmul(
                            st2[:, 512:1024],
                            khat[64:128, kt * 128:(kt + 1) * 128],
                            qhat[64:128, qc * 512:(qc + 1) * 512],
                            start=True, stop=True, tile_position=(64, 0))
                        pt = ptpool.tile([128, 1024], dt.bfloat16, tag="pt")
                        nc.scalar.activation(pt[:], st2[:], AF.Exp, scale=0.125)
                        t = kt - 4 * qc
                        if t >= 0:
                            msl = strip[:, 384 - 128 * t: 896 - 128 * t]
                            nc.vector.tensor_mul(pt[:, 0:512], pt[:, 0:512], msl)
                            nc.vector.tensor_mul(pt[:, 512:1024], pt[:, 512:1024], msl)
                        vt = v_tiles[kt]
                        nc.tensor.matmul(
                            av[0:64, :], vt[:, 128 * p: 128 * p + 64],
                            pt[:, 0:512],
                            start=(kt == 0), stop=(kt == nkt - 1),
                            tile_position=(0, 0))
                        nc.tensor.matmul(
                            av[64:128, :], vt[:, 128 * p + 64: 128 * p + 128],
                            pt[:, 512:1024],
                            start=(kt == 0), stop=(kt == nkt - 1),
                            tile_position=(0, 64))
                        nc.tensor.matmul(
                            zz[0:1, :], ones1[:], pt[:, 0:512],
                            start=(kt == 0), stop=(kt == nkt - 1),
                            tile_position=(0, 0))
                        nc.tensor.matmul(
                            zz[32:33, :], ones1[:], pt[:, 512:1024],
                            start=(kt == 0), stop=(kt == nkt - 1),
                            tile_position=(0, 32))
                    rz_a = tmp.tile([1, 512], dt.float32, tag="rz_a")
                    rz_b = tmp.tile([1, 512], dt.float32, tag="rz_b")
                    zcp = tmp.tile([1, 512], dt.float32, tag="zcp")
                    nc.vector.reciprocal_approx_fast(rz_a[:], zz[0:1, :])
                    nc.scalar.copy(zcp[:], zz[32:33, :])
                    nc.vector.reciprocal_approx_fast(rz_b[:], zcp[:])
                    rzb = tmp.tile([128, 512], dt.float32, tag="rzb")
                    rzbb = tmp.tile([64, 512], dt.float32, tag="rzbb")
                    nc.gpsimd.partition_broadcast(rzb[0:64, :], rz_a[:])
                    nc.gpsimd.partition_broadcast(rzbb[:], rz_b[:])
                    nc.sync.dma_start(out=rzb[64:128, :], in_=rzbb[:])
                    nc.vector.tensor_mul(ot[:, qc * 512:(qc + 1) * 512],
                                         av[:], rzb[:])

            if DEBUG_TAPS:
                for scd in range(NSC):
                    tf = tmp.tile([128, 512], dt.float32, tag="dbgf")
                    nc.vector.tensor_copy(tf[:], ot_tiles[0][:, scd*512:(scd+1)*512])
                    nc.sync.dma_start(out=dbg["d_ot"][:, scd*512:(scd+1)*512], in_=tf[:])
            # ---- stage C: o_proj partial ----------------------------------
            for st in range(NST):
                for ec in range(2):
                    o_ps = psp.tile([128, 512], dt.float32, tag="ps")
                    for ct in range(2):
                        nc.tensor.matmul(
                            o_ps[:],
                            ot_tiles[ct][:, st * 128:(st + 1) * 128],
                            wo_sb[:, ct * H + ec * 512: ct * H + ec * 512 + 512],
                            start=(ct == 0), stop=(ct == 1))
                    o_sb = tmp.tile([128, 512], dt.float32, tag="osb")
                    nc.vector.tensor_copy(o_sb[:], o_ps[:])
                    nc.sync.dma_start(
                        out=out_d[st * 128:(st + 1) * 128, ec * 512:(ec + 1) * 512],
                        in_=o_sb[:])

    nc.finalize()
    return nc


def host_prep(hidden_states, rope_cos, rope_sin, W_qkv, W_o, gamma_q, gamma_k, S):
    """Build the 8 per-core input maps (bf16)."""
    hidden_states = np.asarray(hidden_states, np.float32)
    rope_cos = np.asarray(rope_cos, np.float32)
    rope_sin = np.asarray(rope_sin, np.float32)
    W_qkv = np.asarray(W_qkv, np.float32)
    W_o = np.asarray(W_o, np.float32)
    gamma_q = np.asarray(gamma_q, np.float32)
    gamma_k = np.asarray(gamma_k, np.float32)

    cos_t = np.ascontiguousarray(rope_cos[0].T)  # [64, S]
    sin_t = np.ascontiguousarray(rope_sin[0].T)
    sgn = np.where(np.arange(HD) < HD // 2, -1.0, 1.0).astype(np.float32)
    shift_idx = (np.arange(HD) + HD // 2) % HD

    def rope_consts(gamma):
        c = gamma[:, None] * cos_t                      # [64, S]
        s = (sgn * gamma[shift_idx])[:, None] * sin_t   # [64, S]
        return (np.concatenate([c, c], 0).astype(BF16),
                np.concatenate([s, s], 0).astype(BF16))

    cq, sq = rope_consts(gamma_q)
    ck, sk = rope_consts(gamma_k)

    in_maps = []
    for core in range(NCORES):
        b, g = core // 4, core % 4
        h0 = g * HEADS_PER_CORE * HD  # column offset, 256 per group
        in_maps.append({
            "xt": np.ascontiguousarray(hidden_states[b].T).astype(BF16),
            "wq": W_qkv[:, h0:h0 + 256].astype(BF16),
            "wk": W_qkv[:, H + h0:H + h0 + 256].astype(BF16),
            "wv": W_qkv[:, 2 * H + h0:2 * H + h0 + 256].astype(BF16),
            "wo": W_o[h0:h0 + 256, :].astype(BF16),
            "cq": cq, "sq": sq, "ck": ck, "sk": sk,
        })
    return in_maps


_NC_CACHE = {}


def run(inputs, S=4096, trace=False):
    from concourse.bass_utils import run_bass_kernel_spmd
    if S not in _NC_CACHE:
        _NC_CACHE[S] = build(S)
    nc = _NC_CACHE[S]
    in_maps = host_prep(S=S, **inputs)
    res = run_bass_kernel_spmd(nc, in_maps, list(range(NCORES)), trace=trace)
    B = 2
    out = np.zeros((B, S, H), np.float32)
    for b in range(B):
        acc = res.results[4 * b]["out"].astype(np.float32)
        for g in range(1, 4):
            acc = acc + res.results[4 * b + g]["out"]
        out[b] = acc
    return out, res


def kernel(**inputs):
    out, _ = run(inputs, S=4096, trace=False)
    return out
